# revision 26
# baseline (speedup 1.0000x reference)
"""ClassAttention kernel for 8x TRN2 NeuronCores (Bass/Tile).

Problem (hardcoded): x[16, 2049, 1024], w_qkv[3072, 1024], w_proj[1024, 1024],
b_proj[1024].  Reference computes qkv projection, class-token attention
(only query position 0 attends), projection of the class token, and returns
concat([cls_tok, x[:, 1:]], axis=1).

Only output row 0 is computed; rows 1.. are x passthrough (host copy).

Algebraic restructure (same math, far fewer FLOPs), split host/device:
    host:   q0[b]         = x[b,0] @ Wq^T
            wfold[b,h,:]  = 32 * SCALE * q0[b,h,:] @ Wk_h    (x32: fp8 range)
    device: logits[b,h,s] = sum_d x[b,s,d] * wfold[b,h,d]     (matmul over d)
            ex[b,h,s]     = exp(logits / 32)                  (no max-shift,
                            logits ~ N(0,1): exp cannot overflow fp32)
            sums[b,h]     = sum_s ex  (via ACT accum, 5 chunks)
            xa[b,h,d]     = sum_s ex[b,h,s] * x[b,s,d]        (matmul over s)
    host:   attn = ex/sum;  cls = per-head xa @ Wv_h^T / sum;  out0 = cls @ Wp^T + b

s is padded 2049 -> 2176 with x=0: pad logits are exactly 0, exp(0)=1, so
device sums are exact + 127.0 (host subtracts); pad rows of xn are 0 so xa
is unaffected.

All matmul operands are fp8 e4m3; the big matmuls run perf_mode=DoubleRow
with the two contraction k-tiles byte-interleaved in the moving operand
(j-innermost) so the PE streams 2 fp8 per partition-cycle.

Sharding: data-parallel over batch, 2 per core.  x is shipped twice (both
contraction layouts) in fp8, in partition-contiguous chunks sized and
ordered to pipeline with PE consumption: xt b0, xt b1, xn b0, xn b1.
"""

import os
import numpy as np
import ml_dtypes

FP8 = ml_dtypes.float8_e4m3

B, S, D, H, E = 16, 2049, 1024, 16, 64
SCALE = E ** -0.5
WF_BOOST = 32.0           # host-side wfold scale so fp8e4 keeps precision
NCORES = 8
BL = B // NCORES          # batches per core = 2
ST = 17                   # s-tiles of 128 (padded)
SP = ST * 128             # 2176 padded sequence
XW = ST * 1024            # 17408 bytes per partition for each x stream

# logits s-chunks: 4x512 + the single real s=2048 column (no shipped pads)
SC_W = [512, 512, 512, 512, 1]
SC_OFF = [0, 512, 1024, 1536, 2048]
XT_OFF = [0, 4096, 8192, 12288, 16384]
XTW = 16392               # xt tile bytes per partition
# xn DMA chunks in pair-block columns (pairs of s-tiles, then the s=2048 tail)
XN_GRP = [(0, 4096), (4096, 8192), (8192, 12288), (12288, 16384),
          (16384, 17408)]

N_WARM = int(os.environ.get("K_WARM", "44"))

_cached = {}


def _kernel_body(ctx, tc):
    import concourse.bass as bass
    from concourse import mybir

    nc = tc.nc
    dt = mybir.dt
    AF = mybir.ActivationFunctionType
    DR = mybir.MatmulPerfMode.DoubleRow

    xt_d = nc.dram_tensor("xt", (BL * 128, XTW), dt.float8e4, kind="ExternalInput").ap()
    xn_d = nc.dram_tensor("xn", (BL * 128, XW), dt.float8e4, kind="ExternalInput").ap()
    wf_d = nc.dram_tensor("wf", (128, BL * 128), dt.float8e4, kind="ExternalInput").ap()
    id_d = nc.dram_tensor("ident", (16, 16), dt.float8e4, kind="ExternalInput").ap()
    out_d = nc.dram_tensor("out", (BL * 16, 1032), dt.float32, kind="ExternalOutput").ap()

    cpool = ctx.enter_context(tc.tile_pool(name="const", bufs=1))
    xpool = ctx.enter_context(tc.tile_pool(name="x", bufs=1))
    spool = ctx.enter_context(tc.tile_pool(name="sm", bufs=1))

    # PSUM: DoubleRow outputs must start at partition 0, so each 512-wide
    # logits chunk needs its own bank: log0-3 (4; the tiny s=2048 chunk
    # shares bank log0 at partition 32 in normal mode) + tr (2) + xa (2) = 8.
    ps_log = ctx.enter_context(tc.tile_pool(name="pslog", bufs=1, space="PSUM"))
    ps_tr = ctx.enter_context(tc.tile_pool(name="pstr", bufs=2, space="PSUM"))
    ps_xa = ctx.enter_context(tc.tile_pool(name="psxa", bufs=2, space="PSUM"))

    # --- persistent x tiles ---
    xt_sb = [xpool.tile([128, XTW], dt.float8e4, tag=f"xt{b}", name=f"xt{b}")
             for b in range(BL)]
    xn_sb = [xpool.tile([128, XW], dt.float8e4, tag=f"xn{b}", name=f"xn{b}")
             for b in range(BL)]

    wf_sb = cpool.tile([128, BL * 128], dt.float8e4, tag="wf")
    id_sb = cpool.tile([16, 16], dt.float8e4, tag="ident")
    warm_sb = cpool.tile([128, 128], dt.bfloat16, tag="warm")
    nc.vector.memset(warm_sb[:], 0.0)

    def load_xt_chunk(b, sc):
        w = SC_W[sc] * 8
        off = XT_OFF[sc]
        nc.sync.dma_start(xt_sb[b][:, off:off + w],
                          xt_d[b * 128:(b + 1) * 128, off:off + w])

    def load_xn_chunk(b, g):
        c0, c1 = XN_GRP[g]
        nc.sync.dma_start(xn_sb[b][:, c0:c1],
                          xn_d[b * 128:(b + 1) * 128, c0:c1])

    # DMA stream in consumption order; the small s=2048 tails ship early so
    # the late xa accumulations never wait on a last-in-queue straggler, and
    # xn b0 interleaves with xt b1 so xa b0 fills logits-b1 stall gaps.
    nc.sync.dma_start(wf_sb[:], wf_d)
    for sc in range(5):
        load_xt_chunk(0, sc)
    nc.sync.dma_start(id_sb[:], id_d)
    load_xn_chunk(0, 4)
    load_xn_chunk(1, 4)
    for sc in range(4):
        load_xt_chunk(1, sc)
        load_xn_chunk(0, sc)
    load_xt_chunk(1, 4)
    for g in range(4):
        load_xn_chunk(1, g)

    # --- output staging ---
    out_sb = [spool.tile([16, 1032], dt.float32, tag=f"out{b}", name=f"out{b}")
              for b in range(BL)]
    expb = [spool.tile([16, SP], dt.float8e4, tag=f"exp{b}", name=f"exp{b}")
            for b in range(BL)]
    atT = [spool.tile([128, ST * 16], dt.float8e4, tag=f"atT{b}", name=f"atT{b}")
           for b in range(BL)]
    # s-pad region of exp is never written by ACT (nothing is shipped for
    # it): zero it so pad attn is 0, not uninitialized (possibly NaN) bytes.
    for b in range(BL):
        nc.vector.memset(expb[b][:, 2049:], 0.0)

    # --- PE warm-up: matmuls on zeros so real work starts at 2.4GHz ---
    for w in range(N_WARM):
        ps = ps_log.tile([128, 512], dt.float32, tag=f"log{w % 4}", name=f"warm{w}")
        nc.tensor.matmul(ps[:, :128], warm_sb[:], warm_sb[:], start=True, stop=True)

    log_tiles = [{}, {}]

    def emit_logits_chunk(b, sc):
        """DoubleRow: 4 passes of byte-interleaved d-block pairs per s-chunk.
        The single-column s=2048 chunk runs in normal mode into bank log0
        at partition 32 (regions within the same tile, so no false WAR)."""
        w = SC_W[sc]
        if sc == 4:
            out = log_tiles[b][0][32:48, 0:1]
            for d8 in range(8):
                nc.tensor.matmul(
                    out,
                    wf_sb[:, (b * 8 + d8) * 16:(b * 8 + d8 + 1) * 16],
                    xt_sb[b][:, XT_OFF[4] + d8: XT_OFF[4] + d8 + 1],
                    start=(d8 == 0), stop=(d8 == 7),
                )
        else:
            tile = ps_log.tile([128, 512], dt.float32, tag=f"log{sc}",
                               name=f"log{sc}_{b}")
            log_tiles[b][sc] = tile
            out = tile[0:16, :w]
            for q in range(4):
                lhs = wf_sb[:, (b * 8 + 2 * q) * 16:(b * 8 + 2 * q + 2) * 16]
                rhs = xt_sb[b][:, XT_OFF[sc] + 2 * q * w: XT_OFF[sc] + (2 * q + 2) * w]
                nc.tensor.matmul(
                    out,
                    lhs.rearrange("p (two h) -> p two h", two=2),
                    rhs.rearrange("p (n two) -> p two n", two=2),
                    start=(q == 0), stop=(q == 3),
                    perf_mode=DR,
                )
        nc.scalar.activation(
            expb[b][:, SC_OFF[sc]:SC_OFF[sc] + w], out,
            AF.Exp, bias=0.0, scale=1.0 / WF_BOOST)
        if sc == 4:
            # one row-sum over the fully written exp row (pads are zero);
            # using the quantized weights keeps sums consistent with xa.
            nc.vector.tensor_reduce(
                out_sb[b][:, 1024:1025], expb[b][:, :],
                axis=mybir.AxisListType.X, op=mybir.AluOpType.add)

    # transpose groups: 4 s-tiles each (last group: st16 alone)
    TR_G = [(0, 4), (4, 8), (8, 12), (12, 16), (16, 17)]
    tr_banks = {}

    def emit_tr_group(b, g):
        """[16,128] slices -> [128,16] atT cols; fp8 transpose needs dst step 2."""
        st0, st1 = TR_G[g]
        bank = ps_tr.tile([128, ST * 32], dt.float8e4, tag="tr", name=f"tr{b}_{g}")
        for st in range(st0, st1):
            nc.tensor.transpose(bank[:, st * 32:(st + 1) * 32:2],
                                expb[b][:, st * 128:(st + 1) * 128],
                                id_sb[:, :])
        nc.vector.tensor_copy(atT[b][:, st0 * 16:st1 * 16],
                              bank[:, st0 * 32:st1 * 32:2])

    def make_xa_accs(b):
        # b1's accumulators live in the (by then dead) logits banks so they
        # never WAR-wait on b0's copies.
        if b == 0:
            return [ps_xa.tile([16, 512], dt.float32, tag="xa", name=f"xa0_{c}")
                    for c in range(2)]
        return [ps_log.tile([16, 512], dt.float32, tag=f"log{c}",
                            name=f"xa1_{c}") for c in range(2)]

    def emit_xa_tail(b, accs):
        # s=2048 term first: its (early-shipped) data never gates the end
        for c in range(2):
            nc.tensor.matmul(
                accs[c][:],
                atT[b][:, 16 * 16: 17 * 16],
                xn_sb[b][:, 16384 + c * 512: 16384 + (c + 1) * 512],
                start=True, stop=False,
            )

    def emit_xa_pair(b, accs, q, c):
        lhs = (atT[b][:, 2 * q * 16: (2 * q + 2) * 16]
               .rearrange("p (two h) -> p two h", two=2))
        rhs = (xn_sb[b][:, q * 2048 + c * 1024: q * 2048 + (c + 1) * 1024]
               .rearrange("p (n two) -> p two n", two=2))
        nc.tensor.matmul(
            accs[c][:], lhs, rhs,
            start=False, stop=(q == 7),
            perf_mode=DR,
        )

    def emit_xa_pairs(b, accs, q0, q1):
        for q in range(q0, q1):
            for c in range(2):
                emit_xa_pair(b, accs, q, c)

    def emit_xa_last(b, accs):
        # close + copy c0 while c1's last accumulation still runs
        emit_xa_pair(b, accs, 7, 0)
        nc.vector.tensor_copy(out_sb[b][:, 0:512], accs[0][:])
        emit_xa_pair(b, accs, 7, 1)
        nc.vector.tensor_copy(out_sb[b][:, 512:1024], accs[1][:])

    # --- PE program: logits b0; logits b1 with b0's transposes filling its
    # DMA-stall gaps; then b1's transposes interleaved into xa b0; xa b1. ---
    for sc in range(5):
        emit_logits_chunk(0, sc)
    for sc in range(5):
        emit_logits_chunk(1, sc)
        emit_tr_group(0, sc)
    accs0 = make_xa_accs(0)
    emit_tr_group(1, 0)
    emit_xa_tail(0, accs0)
    emit_xa_pairs(0, accs0, 0, 2)
    emit_tr_group(1, 1)
    emit_xa_pairs(0, accs0, 2, 4)
    emit_tr_group(1, 2)
    emit_xa_pairs(0, accs0, 4, 6)
    emit_tr_group(1, 3)
    emit_xa_pairs(0, accs0, 6, 7)
    emit_tr_group(1, 4)
    emit_xa_last(0, accs0)
    accs1 = make_xa_accs(1)
    emit_xa_tail(1, accs1)
    emit_xa_pairs(1, accs1, 0, 7)
    emit_xa_last(1, accs1)

    for b in range(BL):
        nc.sync.dma_start(out_d[b * 16:(b + 1) * 16, :], out_sb[b][:])


def _build():
    if "nc" in _cached:
        return _cached["nc"]
    from contextlib import ExitStack
    import concourse.tile as tile
    from concourse import bacc

    nc = bacc.Bacc("TRN2", target_bir_lowering=False, debug=False,
                   num_devices=NCORES)
    with tile.TileContext(nc) as tc:
        with ExitStack() as ctx:
            _kernel_body(ctx, tc)
    nc.compile()
    _cached["nc"] = nc
    return nc


def _host_prep(x, w_qkv):
    """Per-core input buffers: partition-contiguous, DR k-pairs interleaved."""
    x = np.asarray(x, dtype=np.float32)
    w_qkv = np.asarray(w_qkv, dtype=np.float32)

    w_q, w_k = w_qkv[:D], w_qkv[D:2 * D]
    q0 = x[:, 0, :] @ w_q.T                                   # [B, D]
    wfold = np.einsum("bhe,hed->bhd", q0.reshape(B, H, E),
                      w_k.reshape(H, E, D)) * (SCALE * WF_BOOST)

    xc = x.astype(FP8)                                        # [B, S, D] fp8

    id_dev = np.eye(16, dtype=FP8)
    in_maps = []
    for c in range(NCORES):
        b0 = c * BL
        xt = np.zeros((BL, 128, XTW), dtype=FP8)
        xn = np.zeros((BL, 128, XW), dtype=FP8)
        for bb in range(BL):
            xb = xc[b0 + bb]                                  # [S, D]
            xbT = np.ascontiguousarray(xb.T)                  # [D, S]
            for sc in range(5):
                w = SC_W[sc]
                s0 = SC_OFF[sc]
                s1 = min(s0 + w, S)
                blk = np.zeros((D, w), dtype=FP8)
                blk[:, :s1 - s0] = xbT[:, s0:s1]
                # [D, w] -> (q, j, p, n) -> [p, q, n, j]: j byte-interleaved
                xt[bb, :, XT_OFF[sc]:XT_OFF[sc] + 8 * w] = (
                    blk.reshape(4, 2, 128, w).transpose(2, 0, 3, 1)
                    .reshape(128, 8 * w))
            xnb = np.zeros((SP, D), dtype=FP8)
            xnb[:S] = xb
            # pairs of s-tiles byte-interleaved: [p, P, d, j]
            xn[bb, :, :16384] = (xnb[:2048].reshape(8, 2, 128, D)
                                 .transpose(2, 0, 3, 1).reshape(128, 8 * 2048))
            xn[bb, :, 16384:] = (xnb[2048:].reshape(1, 128, D)
                                 .transpose(1, 0, 2).reshape(128, 1024))
        wf_core = (wfold[b0:b0 + BL].reshape(BL, H, 8, 128)
                   .transpose(3, 0, 2, 1).reshape(128, BL * 128).astype(FP8))
        in_maps.append({
            "xt": xt.reshape(BL * 128, XTW),
            "xn": xn.reshape(BL * 128, XW),
            "wf": np.ascontiguousarray(wf_core),
            "ident": id_dev,
        })
    return x, in_maps


def _host_tail(x, dev_outs, w_qkv, w_proj, b_proj):
    """cls projection + output projection on host from device xa/sums."""
    w_qkv = np.asarray(w_qkv, dtype=np.float32)
    w_proj = np.asarray(w_proj, dtype=np.float32)
    b_proj = np.asarray(b_proj, dtype=np.float32)
    w_v = w_qkv[2 * D:]                                       # [D, D] (he, d)

    out = x.copy()
    for c in range(NCORES):
        dev = np.asarray(dev_outs[c], dtype=np.float32)       # [BL*16, 1032]
        for bb in range(BL):
            xa = dev[bb * 16:(bb + 1) * 16, :1024]            # [H, D] unnorm
            sums = dev[bb * 16:(bb + 1) * 16, 1024]
            attn_x = xa / sums[:, None]                       # [H, D]
            cls = np.einsum("hd,hed->he", attn_x,
                            w_v.reshape(H, E, D))             # [H, E]
            out[c * BL + bb, 0, :] = cls.reshape(D) @ w_proj.T + b_proj
    return out


def _run(x, w_qkv, w_proj, b_proj, trace=False):
    from concourse import bass_utils
    try:
        import jax
        jax.config.update("jax_compilation_cache_dir", "/tmp/jax_pjrt_cache")
        jax.config.update("jax_persistent_cache_min_compile_time_secs", 2.0)
    except Exception:
        pass

    nc = _build()
    x, in_maps = _host_prep(x, w_qkv)
    res = bass_utils.run_bass_kernel_spmd(
        nc, in_maps, core_ids=list(range(NCORES)), trace=trace)

    dev_outs = [res.results[c]["out"] for c in range(NCORES)]
    out = _host_tail(x, dev_outs, w_qkv, w_proj, b_proj)
    return out, res


def kernel(x, w_qkv, w_proj, b_proj):
    out, _ = _run(x, w_qkv, w_proj, b_proj, trace=False)
    return out


# revision 30
# speedup vs baseline: 1.1246x; 1.1246x over previous
"""ClassAttention kernel for 8x TRN2 NeuronCores (Bass/Tile).

Problem (hardcoded): x[16, 2049, 1024], w_qkv[3072, 1024], w_proj[1024, 1024],
b_proj[1024].  Reference computes qkv projection, class-token attention
(only query position 0 attends), projection of the class token, and returns
concat([cls_tok, x[:, 1:]], axis=1).

Only output row 0 is computed; rows 1.. are x passthrough (host copy).

Algebraic restructure (same math, far fewer FLOPs), split host/device:
    host:   q0[b]         = x[b,0] @ Wq^T
            wfold[b,h,:]  = 32 * SCALE * q0[b,h,:] @ Wk_h    (x32: fp8 range)
    device: logits[b,h,s] = sum_d x[b,s,d] * wfold[b,h,d]     (matmul over d)
            ex[b,h,s]     = exp(logits / 32)                  (no max-shift,
                            logits ~ N(0,1): exp cannot overflow fp32)
            sums[b,h]     = sum_s ex  (via ACT accum, 5 chunks)
            xa[b,h,d]     = sum_s ex[b,h,s] * x[b,s,d]        (matmul over s)
    host:   attn = ex/sum;  cls = per-head xa @ Wv_h^T / sum;  out0 = cls @ Wp^T + b

s is padded 2049 -> 2176 with x=0: pad logits are exactly 0, exp(0)=1, so
device sums are exact + 127.0 (host subtracts); pad rows of xn are 0 so xa
is unaffected.

All matmul operands are fp8 e4m3; the big matmuls run perf_mode=DoubleRow
with the two contraction k-tiles byte-interleaved in the moving operand
(j-innermost) so the PE streams 2 fp8 per partition-cycle.

Sharding: data-parallel over batch, 2 per core.  x is shipped twice (both
contraction layouts) in fp8, in partition-contiguous chunks sized and
ordered to pipeline with PE consumption: xt b0, xt b1, xn b0, xn b1.
"""

import os
import numpy as np
import ml_dtypes

FP8 = ml_dtypes.float8_e4m3

B, S, D, H, E = 16, 2049, 1024, 16, 64
SCALE = E ** -0.5
WF_BOOST = 32.0           # host-side wfold scale so fp8e4 keeps precision
NCORES = 8
BL = B // NCORES          # batches per core = 2
ST = 17                   # s-tiles of 128 (padded)
SP = ST * 128             # 2176 padded sequence
XW = ST * 1024            # 17408 bytes per partition for each x stream

# logits s-chunks: 4x512 + the single real s=2048 column (no shipped pads)
SC_W = [512, 512, 512, 512, 1]
SC_OFF = [0, 512, 1024, 1536, 2048]
XT_OFF = [0, 4096, 8192, 12288, 16384]
XTW = 16392               # xt tile bytes per partition
# xn DMA chunks in pair-block columns (pairs of s-tiles, then the s=2048 tail)
XN_GRP = [(0, 4096), (4096, 8192), (8192, 12288), (12288, 16384),
          (16384, 17408)]

N_WARM = int(os.environ.get("K_WARM", "44"))

_cached = {}


def _kernel_body(ctx, tc):
    import concourse.bass as bass
    from concourse import mybir

    nc = tc.nc
    dt = mybir.dt
    AF = mybir.ActivationFunctionType
    DR = mybir.MatmulPerfMode.DoubleRow

    xt_d = nc.dram_tensor("xt", (BL * 128, XTW), dt.float8e4, kind="ExternalInput").ap()
    xn_d = nc.dram_tensor("xn", (BL * 128, XW), dt.float8e4, kind="ExternalInput").ap()
    wf_d = nc.dram_tensor("wf", (128, BL * 128), dt.float8e4, kind="ExternalInput").ap()
    id_d = nc.dram_tensor("ident", (16, 16), dt.float8e4, kind="ExternalInput").ap()
    out_d = nc.dram_tensor("out", (BL * 16, 1032), dt.float32, kind="ExternalOutput").ap()

    cpool = ctx.enter_context(tc.tile_pool(name="const", bufs=1))
    xpool = ctx.enter_context(tc.tile_pool(name="x", bufs=1))
    spool = ctx.enter_context(tc.tile_pool(name="sm", bufs=1))

    # PSUM: DoubleRow outputs must start at partition 0, so each 512-wide
    # logits chunk needs its own bank: log0-3 (4; the tiny s=2048 chunk
    # shares bank log0 at partition 32 in normal mode) + tr (2) + xa (2) = 8.
    ps_log = ctx.enter_context(tc.tile_pool(name="pslog", bufs=1, space="PSUM"))
    ps_tr = ctx.enter_context(tc.tile_pool(name="pstr", bufs=2, space="PSUM"))
    ps_xa = ctx.enter_context(tc.tile_pool(name="psxa", bufs=2, space="PSUM"))

    # --- persistent x tiles ---
    xt_sb = [xpool.tile([128, XTW], dt.float8e4, tag=f"xt{b}", name=f"xt{b}")
             for b in range(BL)]
    xn_sb = [xpool.tile([128, XW], dt.float8e4, tag=f"xn{b}", name=f"xn{b}")
             for b in range(BL)]

    wf_sb = cpool.tile([128, BL * 128], dt.float8e4, tag="wf")
    id_sb = cpool.tile([16, 16], dt.float8e4, tag="ident")
    warm_sb = cpool.tile([128, 128], dt.bfloat16, tag="warm")
    nc.vector.memset(warm_sb[:], 0.0)

    def load_xt_chunk(b, sc):
        w = SC_W[sc] * 8
        off = XT_OFF[sc]
        nc.sync.dma_start(xt_sb[b][:, off:off + w],
                          xt_d[b * 128:(b + 1) * 128, off:off + w])

    def load_xn_chunk(b, g):
        c0, c1 = XN_GRP[g]
        nc.sync.dma_start(xn_sb[b][:, c0:c1],
                          xn_d[b * 128:(b + 1) * 128, c0:c1])

    # DMA stream in consumption order; the small s=2048 tails ship early so
    # the late xa accumulations never wait on a last-in-queue straggler, and
    # xn b0 ships BEFORE xt b1 so xa b0 + tr b0 fill logits-b0/b1 stall gaps
    # (keeping the PE busy enough that HAM never re-throttles the clock).
    nc.sync.dma_start(wf_sb[:], wf_d)
    for sc in range(5):
        load_xt_chunk(0, sc)
    nc.sync.dma_start(id_sb[:], id_d)
    load_xn_chunk(0, 4)
    load_xn_chunk(1, 4)
    for g in range(4):
        load_xn_chunk(0, g)
    for sc in range(5):
        load_xt_chunk(1, sc)
    for g in range(4):
        load_xn_chunk(1, g)

    # --- output staging ---
    out_sb = [spool.tile([16, 1032], dt.float32, tag=f"out{b}", name=f"out{b}")
              for b in range(BL)]
    expb = [spool.tile([16, SP], dt.float8e4, tag=f"exp{b}", name=f"exp{b}")
            for b in range(BL)]
    atT = [spool.tile([128, ST * 16], dt.float8e4, tag=f"atT{b}", name=f"atT{b}")
           for b in range(BL)]
    # s-pad region of exp is never written by ACT (nothing is shipped for
    # it): zero it so pad attn is 0, not uninitialized (possibly NaN) bytes.
    for b in range(BL):
        nc.vector.memset(expb[b][:, 2049:], 0.0)

    # --- PE warm-up: matmuls on zeros so real work starts at 2.4GHz ---
    for w in range(N_WARM):
        ps = ps_log.tile([128, 512], dt.float32, tag=f"log{w % 4}", name=f"warm{w}")
        nc.tensor.matmul(ps[:, :128], warm_sb[:], warm_sb[:], start=True, stop=True)

    log_tiles = [{}, {}]

    def emit_logits_chunk(b, sc):
        """DoubleRow: 4 passes of byte-interleaved d-block pairs per s-chunk.
        The single-column s=2048 chunk runs in normal mode into bank log0
        at partition 32 (regions within the same tile, so no false WAR)."""
        w = SC_W[sc]
        if sc == 4:
            out = log_tiles[b][0][32:48, 0:1]
            for d8 in range(8):
                nc.tensor.matmul(
                    out,
                    wf_sb[:, (b * 8 + d8) * 16:(b * 8 + d8 + 1) * 16],
                    xt_sb[b][:, XT_OFF[4] + d8: XT_OFF[4] + d8 + 1],
                    start=(d8 == 0), stop=(d8 == 7),
                )
        else:
            tile = ps_log.tile([128, 512], dt.float32, tag=f"log{sc}",
                               name=f"log{sc}_{b}")
            log_tiles[b][sc] = tile
            out = tile[0:16, :w]
            for q in range(4):
                lhs = wf_sb[:, (b * 8 + 2 * q) * 16:(b * 8 + 2 * q + 2) * 16]
                rhs = xt_sb[b][:, XT_OFF[sc] + 2 * q * w: XT_OFF[sc] + (2 * q + 2) * w]
                nc.tensor.matmul(
                    out,
                    lhs.rearrange("p (two h) -> p two h", two=2),
                    rhs.rearrange("p (n two) -> p two n", two=2),
                    start=(q == 0), stop=(q == 3),
                    perf_mode=DR,
                )
        nc.scalar.activation(
            expb[b][:, SC_OFF[sc]:SC_OFF[sc] + w], out,
            AF.Exp, bias=0.0, scale=1.0 / WF_BOOST)
        if sc == 4:
            # one row-sum over the fully written exp row (pads are zero);
            # using the quantized weights keeps sums consistent with xa.
            nc.vector.tensor_reduce(
                out_sb[b][:, 1024:1025], expb[b][:, :],
                axis=mybir.AxisListType.X, op=mybir.AluOpType.add)

    # transpose groups: 4 s-tiles each (last group: st16 alone)
    TR_G = [(0, 4), (4, 8), (8, 12), (12, 16), (16, 17)]
    tr_banks = {}

    def emit_tr_group(b, g):
        """[16,128] slices -> [128,16] atT cols; fp8 transpose needs dst step 2."""
        st0, st1 = TR_G[g]
        bank = ps_tr.tile([128, ST * 32], dt.float8e4, tag="tr", name=f"tr{b}_{g}")
        for st in range(st0, st1):
            nc.tensor.transpose(bank[:, st * 32:(st + 1) * 32:2],
                                expb[b][:, st * 128:(st + 1) * 128],
                                id_sb[:, :])
        nc.vector.tensor_copy(atT[b][:, st0 * 16:st1 * 16],
                              bank[:, st0 * 32:st1 * 32:2])

    def make_xa_accs(b):
        # b1's accumulators live in the (by then dead) logits banks so they
        # never WAR-wait on b0's copies.
        if b == 0:
            return [ps_xa.tile([16, 512], dt.float32, tag="xa", name=f"xa0_{c}")
                    for c in range(2)]
        return [ps_log.tile([16, 512], dt.float32, tag=f"log{c}",
                            name=f"xa1_{c}") for c in range(2)]

    def emit_xa_tail(b, accs):
        # s=2048 term (tiny; its data shipped early, transposes by now done)
        for c in range(2):
            nc.tensor.matmul(
                accs[c][:],
                atT[b][:, 16 * 16: 17 * 16],
                xn_sb[b][:, 16384 + c * 512: 16384 + (c + 1) * 512],
                start=False, stop=False,
            )

    def emit_xa_pair(b, accs, q, c):
        lhs = (atT[b][:, 2 * q * 16: (2 * q + 2) * 16]
               .rearrange("p (two h) -> p two h", two=2))
        rhs = (xn_sb[b][:, q * 2048 + c * 1024: q * 2048 + (c + 1) * 1024]
               .rearrange("p (n two) -> p two n", two=2))
        nc.tensor.matmul(
            accs[c][:], lhs, rhs,
            start=(q == 0), stop=(q == 7),
            perf_mode=DR,
        )

    def emit_xa_pairs(b, accs, q0, q1):
        for q in range(q0, q1):
            for c in range(2):
                emit_xa_pair(b, accs, q, c)

    def emit_xa_last(b, accs):
        # close + copy c0 while c1's last accumulation still runs
        emit_xa_pair(b, accs, 7, 0)
        nc.vector.tensor_copy(out_sb[b][:, 0:512], accs[0][:])
        emit_xa_pair(b, accs, 7, 1)
        nc.vector.tensor_copy(out_sb[b][:, 512:1024], accs[1][:])

    # --- PE program: logits b0 with tr b0 + xa b0 as gap filler (their data
    # arrives during it); then logits b1 with tr b1 + xa b1 as filler. ---
    for sc in range(5):
        emit_logits_chunk(0, sc)
    def emit_xa_block(b):
        accs = make_xa_accs(b)
        emit_tr_group(b, 0)
        emit_xa_pairs(b, accs, 0, 2)
        emit_tr_group(b, 1)
        emit_xa_pairs(b, accs, 2, 4)
        emit_tr_group(b, 2)
        emit_xa_pairs(b, accs, 4, 6)
        emit_tr_group(b, 3)
        emit_tr_group(b, 4)
        emit_xa_tail(b, accs)
        emit_xa_pairs(b, accs, 6, 7)
        emit_xa_last(b, accs)

    emit_xa_block(0)
    for sc in range(5):
        emit_logits_chunk(1, sc)
    emit_xa_block(1)

    for b in range(BL):
        nc.sync.dma_start(out_d[b * 16:(b + 1) * 16, :], out_sb[b][:])


def _build():
    if "nc" in _cached:
        return _cached["nc"]
    from contextlib import ExitStack
    import concourse.tile as tile
    from concourse import bacc

    nc = bacc.Bacc("TRN2", target_bir_lowering=False, debug=False,
                   num_devices=NCORES)
    with tile.TileContext(nc) as tc:
        with ExitStack() as ctx:
            _kernel_body(ctx, tc)
    nc.compile()
    _cached["nc"] = nc
    return nc


def _host_prep(x, w_qkv):
    """Per-core input buffers: partition-contiguous, DR k-pairs interleaved."""
    x = np.asarray(x, dtype=np.float32)
    w_qkv = np.asarray(w_qkv, dtype=np.float32)

    w_q, w_k = w_qkv[:D], w_qkv[D:2 * D]
    q0 = x[:, 0, :] @ w_q.T                                   # [B, D]
    wfold = np.einsum("bhe,hed->bhd", q0.reshape(B, H, E),
                      w_k.reshape(H, E, D)) * (SCALE * WF_BOOST)

    xc = x.astype(FP8)                                        # [B, S, D] fp8

    id_dev = np.eye(16, dtype=FP8)
    in_maps = []
    for c in range(NCORES):
        b0 = c * BL
        xt = np.zeros((BL, 128, XTW), dtype=FP8)
        xn = np.zeros((BL, 128, XW), dtype=FP8)
        for bb in range(BL):
            xb = xc[b0 + bb]                                  # [S, D]
            xbT = np.ascontiguousarray(xb.T)                  # [D, S]
            for sc in range(5):
                w = SC_W[sc]
                s0 = SC_OFF[sc]
                s1 = min(s0 + w, S)
                blk = np.zeros((D, w), dtype=FP8)
                blk[:, :s1 - s0] = xbT[:, s0:s1]
                # [D, w] -> (q, j, p, n) -> [p, q, n, j]: j byte-interleaved
                xt[bb, :, XT_OFF[sc]:XT_OFF[sc] + 8 * w] = (
                    blk.reshape(4, 2, 128, w).transpose(2, 0, 3, 1)
                    .reshape(128, 8 * w))
            xnb = np.zeros((SP, D), dtype=FP8)
            xnb[:S] = xb
            # pairs of s-tiles byte-interleaved: [p, P, d, j]
            xn[bb, :, :16384] = (xnb[:2048].reshape(8, 2, 128, D)
                                 .transpose(2, 0, 3, 1).reshape(128, 8 * 2048))
            xn[bb, :, 16384:] = (xnb[2048:].reshape(1, 128, D)
                                 .transpose(1, 0, 2).reshape(128, 1024))
        wf_core = (wfold[b0:b0 + BL].reshape(BL, H, 8, 128)
                   .transpose(3, 0, 2, 1).reshape(128, BL * 128).astype(FP8))
        in_maps.append({
            "xt": xt.reshape(BL * 128, XTW),
            "xn": xn.reshape(BL * 128, XW),
            "wf": np.ascontiguousarray(wf_core),
            "ident": id_dev,
        })
    return x, in_maps


def _host_tail(x, dev_outs, w_qkv, w_proj, b_proj):
    """cls projection + output projection on host from device xa/sums."""
    w_qkv = np.asarray(w_qkv, dtype=np.float32)
    w_proj = np.asarray(w_proj, dtype=np.float32)
    b_proj = np.asarray(b_proj, dtype=np.float32)
    w_v = w_qkv[2 * D:]                                       # [D, D] (he, d)

    out = x.copy()
    for c in range(NCORES):
        dev = np.asarray(dev_outs[c], dtype=np.float32)       # [BL*16, 1032]
        for bb in range(BL):
            xa = dev[bb * 16:(bb + 1) * 16, :1024]            # [H, D] unnorm
            sums = dev[bb * 16:(bb + 1) * 16, 1024]
            attn_x = xa / sums[:, None]                       # [H, D]
            cls = np.einsum("hd,hed->he", attn_x,
                            w_v.reshape(H, E, D))             # [H, E]
            out[c * BL + bb, 0, :] = cls.reshape(D) @ w_proj.T + b_proj
    return out


def _run(x, w_qkv, w_proj, b_proj, trace=False):
    from concourse import bass_utils
    try:
        import jax
        jax.config.update("jax_compilation_cache_dir", "/tmp/jax_pjrt_cache")
        jax.config.update("jax_persistent_cache_min_compile_time_secs", 2.0)
    except Exception:
        pass

    nc = _build()
    x, in_maps = _host_prep(x, w_qkv)
    res = bass_utils.run_bass_kernel_spmd(
        nc, in_maps, core_ids=list(range(NCORES)), trace=trace)

    dev_outs = [res.results[c]["out"] for c in range(NCORES)]
    out = _host_tail(x, dev_outs, w_qkv, w_proj, b_proj)
    return out, res


def kernel(x, w_qkv, w_proj, b_proj):
    out, _ = _run(x, w_qkv, w_proj, b_proj, trace=False)
    return out
